# revision 6
# baseline (speedup 1.0000x reference)
"""Trainium2 Bass kernel for the non-local-attention block (nn_DNL_74234214744693).

Reference computation (B=4, C=64, H=W=64, N=H*W=4096):
    k = conv1x1(x,kw,kb); k_wh = k - mean_j(k)
    q = conv1x1(x,qw,qb); q_wh = q - mean_j(q)
    qk[b,i,j] = sum_c k_wh[b,c,i] q_wh[b,c,j]
    m  = conv1x1(x,mw,mb) -> [B,N];  mm[b,i,j] = m[b,i]*m[b,j]
    f  = softmax(qk, axis=-1) + softmax(mm, axis=0)   # second softmax over BATCH
    y  = einsum('bci,bij->bcj', v, f) + BN(conv1x1(x,ww,wb))

Key algebraic facts used:
  * softmax_j(k_whT q_wh) == softmax_j(k_whT q_raw): the q-mean term is constant
    along j's softmax rows, so only k needs whitening.
  * q-conv fusion: qk[i,j] = sum_c' g[c',i] x_ext[c',j] with
    g = (qw|qb)^T k_wh  (65 x SL), x_ext having a trailing ones row.
    This removes the full q conv + its PSUM->SBUF copies entirely.
  * softmax_j normalizer Z1[i] indexes the contraction dim, so y1 = (v/Z1) @ e1.
  * batch softmax: f2[b] = e2_b * R with e2_b = exp(m_b_i m_b_j), R = 1/sum_b e2_b.
  * m and the k-mean are data-independent 1-D convs -> computed on the host.

Sharding: each of 8 cores owns a 512-row i-slice of the [N,N] maps for ALL 4
batch samples (exp work is perfectly balanced, no duplication, no collectives).
Each core emits a partial y [4,64,4096]; host sums the 8 partials.
The conv+BN residual is folded into the output matmul with weights pre-scaled
by 1/8 (so the host-side sum reconstructs it exactly once).
"""

import functools

import numpy as np
import ml_dtypes

N_CORES = 8
B, C, H, W = 4, 64, 64, 64
N = H * W                 # 4096
SL = N // N_CORES         # 512  rows of the attention map per core
NIT = SL // 128           # 4    128-row tiles per core
NJQ = 4                   # 1024-wide column blocks in phase B
JQ = N // NJQ             # 1024
EPS = 1e-5

BF16 = ml_dtypes.bfloat16


def _build_program():
    import concourse.bass as bass
    import concourse.tile as tile
    from concourse import bacc, mybir

    dt = mybir.dt
    AF = mybir.ActivationFunctionType
    ALU = mybir.AluOpType
    AX = mybir.AxisListType

    nc = bacc.Bacc("TRN2", target_bir_lowering=False, debug=False,
                   enable_asserts=False, num_devices=1)

    # ---------------- DRAM I/O ----------------
    x_ext = nc.dram_tensor("x_ext", [B, C + 1, N], dt.bfloat16, kind="ExternalInput")
    xsl_ext = nc.dram_tensor("xsl_ext", [B, C + 1, SL], dt.bfloat16, kind="ExternalInput")
    kT = nc.dram_tensor("kT", [C + 1, C], dt.bfloat16, kind="ExternalInput")
    qg = nc.dram_tensor("qg", [C, C + 1], dt.bfloat16, kind="ExternalInput")
    vT = nc.dram_tensor("vT", [C + 1, C], dt.bfloat16, kind="ExternalInput")
    wT = nc.dram_tensor("wT", [C + 1, C], dt.bfloat16, kind="ExternalInput")
    negku = nc.dram_tensor("negku", [C, B], dt.float32, kind="ExternalInput")
    msl = nc.dram_tensor("msl", [128, B * NIT], dt.float32, kind="ExternalInput")
    md = nc.dram_tensor("md", [B, N], dt.bfloat16, kind="ExternalInput")
    y_part = nc.dram_tensor("y_part", [B, C, N], dt.float32, kind="ExternalOutput")

    with tile.TileContext(nc) as tc:
        from contextlib import ExitStack

        with ExitStack() as top:
            # ---------- persistent pools ----------
            consts = top.enter_context(tc.tile_pool(name="consts", bufs=1))
            p_vT = top.enter_context(tc.tile_pool(name="p_vT", bufs=B))
            p_v1p = top.enter_context(tc.tile_pool(name="p_v1p", bufs=B))
            p_f1 = top.enter_context(tc.tile_pool(name="p_f1", bufs=B * NIT))

            sb_kT = consts.tile([C + 1, C], dt.bfloat16)
            sb_qg = consts.tile([C, C + 1], dt.bfloat16)
            sb_vT = consts.tile([C + 1, C], dt.bfloat16)
            sb_wT = consts.tile([C + 1, C], dt.bfloat16)
            sb_negku = consts.tile([C, B], dt.float32)
            sb_msl = consts.tile([128, B * NIT], dt.float32)
            nc.sync.dma_start(sb_kT, kT.ap())
            nc.sync.dma_start(sb_qg, qg.ap())
            nc.sync.dma_start(sb_vT, vT.ap())
            nc.sync.dma_start(sb_wT, wT.ap())
            nc.sync.dma_start(sb_negku, negku.ap())
            nc.sync.dma_start(sb_msl, msl.ap())

            # v_T[b][:, it*64:(it+1)*64] is the [128 i, 64 c] tile for row-tile it
            v_T = [p_vT.tile([128, NIT * C], dt.bfloat16, name=f"v_T{b}", tag="v_T") for b in range(B)]
            v1p = [p_v1p.tile([128, NIT * C], dt.bfloat16, name=f"v1p{b}", tag="v1p") for b in range(B)]
            f1 = [[p_f1.tile([128, N], dt.bfloat16, name=f"f1_{b}_{i}", tag="f1") for i in range(NIT)] for b in range(B)]

            # ---------- phase A: per-b convs (tiny) + qk + e1 ----------
            with ExitStack() as ph0:
                p_x = ph0.enter_context(tc.tile_pool(name="p_x", bufs=2))
                p_xsl = ph0.enter_context(tc.tile_pool(name="p_xsl", bufs=2))
                p_kwh = ph0.enter_context(tc.tile_pool(name="p_kwh", bufs=2))
                p_g = ph0.enter_context(tc.tile_pool(name="p_g", bufs=2))
                psP = ph0.enter_context(tc.tile_pool(name="psP", bufs=2, space="PSUM"))
                p_z = ph0.enter_context(tc.tile_pool(name="p_z", bufs=8))

                def dma_phase(b):
                    x_sb = p_x.tile([C + 1, N], dt.bfloat16, name=f"x_sb{b}", tag="x_sb")
                    xsl_sb = p_xsl.tile([C + 1, SL], dt.bfloat16, name=f"xsl_sb{b}", tag="xsl_sb")
                    nc.sync.dma_start(x_sb, x_ext.ap()[b])
                    nc.sync.dma_start(xsl_sb, xsl_ext.ap()[b])
                    return x_sb, xsl_sb

                def conv_phase(b, x_sb, xsl_sb):
                    # one packed psum tile: g' [65,512] | k [64,512] | v_T [128,256]
                    pc = psP.tile([128, 2048], dt.float32, name=f"pc{b}", tag="psP")
                    k_wh = p_kwh.tile([C, SL], dt.bfloat16, name=f"k_wh{b}", tag="k_wh")
                    g_sb = p_g.tile([C + 1, SL], dt.bfloat16, name=f"g_sb{b}", tag="g_sb")

                    nc.tensor.matmul(pc[0:C, 512:1024], sb_kT, xsl_sb,
                                     start=True, stop=True)
                    nc.vector.tensor_scalar(k_wh, pc[0:C, 512:1024],
                                            scalar1=sb_negku[:, b:b + 1],
                                            scalar2=None, op0=ALU.add)
                    nc.tensor.matmul(pc[0:C + 1, 0:512], sb_qg, k_wh,
                                     start=True, stop=True)
                    nc.vector.tensor_copy(g_sb, pc[0:C + 1, 0:512])
                    for it in range(NIT):
                        nc.tensor.matmul(pc[:, 1024 + it * C:1024 + (it + 1) * C],
                                         xsl_sb[:, it * 128:(it + 1) * 128],
                                         sb_vT, start=True, stop=True)
                    nc.vector.tensor_copy(v_T[b], pc[:, 1024:1024 + NIT * C])
                    return k_wh, g_sb

                def qk_phase(b, g_sb, x_sb, its):
                    for it in its:
                        zp = [p_z.tile([128, 1], dt.float32, name=f"zp{j}", tag="zp") for j in range(2)]
                        for jh in range(2):
                            ps_qk = psP.tile([128, 2048], dt.float32, name="ps_qk", tag="psP")
                            for k4 in range(4):
                                j0 = jh * 2048 + k4 * 512
                                nc.tensor.matmul(
                                    ps_qk[:, k4 * 512:(k4 + 1) * 512],
                                    g_sb[:, it * 128:(it + 1) * 128],
                                    x_sb[:, j0:j0 + 512],
                                    start=True, stop=True)
                            nc.scalar.activation(
                                f1[b][it][:, jh * 2048:(jh + 1) * 2048],
                                ps_qk, AF.Exp, accum_out=zp[jh])
                        z1 = p_z.tile([128, 1], dt.float32)
                        rz = p_z.tile([128, 1], dt.float32)
                        nc.vector.tensor_tensor(z1, zp[0], zp[1], op=ALU.add)
                        nc.vector.reciprocal_approx_fast(rz, z1)
                        nc.vector.tensor_scalar_mul(
                            v1p[b][:, it * C:(it + 1) * C],
                            v_T[b][:, it * C:(it + 1) * C], rz)

                # per-b: convs then qk; conv(b+1) emitted before qk(b)'s last
                # tile so its psum slot + DVE copies hide under e1 exps.
                dmas_cur = dma_phase(0)
                conv_cur = conv_phase(0, *dmas_cur)
                for b in range(B):
                    x_cur = dmas_cur[0]
                    if b + 1 < B:
                        dmas_next = dma_phase(b + 1)
                        qk_phase(b, conv_cur[1], x_cur, range(NIT - 1))
                        conv_next = conv_phase(b + 1, *dmas_next)
                        qk_phase(b, conv_cur[1], x_cur, [NIT - 1])
                        conv_cur = conv_next
                        dmas_cur = dmas_next
                    else:
                        qk_phase(b, conv_cur[1], x_cur, range(NIT))

            # ---------- phase B: e2/D/R/f2 + output matmuls ----------
            with ExitStack() as phB:
                psY = phB.enter_context(tc.tile_pool(name="psY", bufs=8, space="PSUM"))
                p_mbc = phB.enter_context(tc.tile_pool(name="p_mbc", bufs=6))
                p_e2 = phB.enter_context(tc.tile_pool(name="p_e2", bufs=8))
                p_s = phB.enter_context(tc.tile_pool(name="p_s", bufs=4))
                p_dr = phB.enter_context(tc.tile_pool(name="p_dr", bufs=1))
                p_rr = phB.enter_context(tc.tile_pool(name="p_rr", bufs=1))
                p_rb = phB.enter_context(tc.tile_pool(name="p_rb", bufs=2))
                p_xw = phB.enter_context(tc.tile_pool(name="p_xw", bufs=4))
                p_out = phB.enter_context(tc.tile_pool(name="p_out", bufs=2))

                for jq in range(NJQ):
                    jsl = slice(jq * JQ, (jq + 1) * JQ)
                    m_bc = []
                    for b in range(B):
                        t = p_mbc.tile([128, JQ], dt.bfloat16, name="m_bc", tag="m_bc")
                        nc.sync.dma_start(t, md.ap()[b:b + 1, jsl].to_broadcast([128, JQ]))
                        m_bc.append(t)
                    x_wx = []
                    for b in range(B):
                        t = p_xw.tile([C + 1, JQ], dt.bfloat16, name="x_wx", tag="x_wx")
                        nc.sync.dma_start(t, x_ext.ap()[b][:, jsl])
                        x_wx.append(t)

                    ps_y = [[psY.tile([C, 512], dt.float32, name=f"ps_y{b}_{h}", tag="ps_y")
                             for h in range(2)] for b in range(B)]
                    # wx residual first: f2-independent, opens each accumulation
                    # group early so it closes right after the last f2 matmul.
                    for b in range(B):
                        for h in range(2):
                            cs = slice(h * 512, (h + 1) * 512)
                            nc.tensor.matmul(ps_y[b][h], sb_wT, x_wx[b][:, cs],
                                             start=True, stop=False)
                    for it in range(NIT):
                        # e2_b = exp(m_i * m_j); D = sum_b e2; f2_b = e2_b / D
                        e2 = [p_e2.tile([128, JQ], dt.bfloat16, name=f"e2_{b}", tag="e2") for b in range(B)]
                        for b in range(B):
                            nc.scalar.activation(e2[b], m_bc[b], AF.Exp,
                                                 scale=sb_msl[:, b * NIT + it:b * NIT + it + 1])
                        s12 = p_s.tile([128, JQ], dt.bfloat16, tag="s12")
                        s34 = p_s.tile([128, JQ], dt.bfloat16, tag="s34")
                        dd = p_dr.tile([128, JQ], dt.float32, tag="dd")
                        rr = p_rr.tile([128, JQ], dt.float32, tag="rr")
                        rrb = p_rb.tile([128, JQ], dt.bfloat16, tag="rrb")
                        nc.vector.tensor_tensor(s12, e2[0], e2[1], op=ALU.add)
                        nc.vector.tensor_tensor(s34, e2[2], e2[3], op=ALU.add)
                        nc.vector.tensor_tensor(dd, s12, s34, op=ALU.add)
                        nc.vector.reciprocal_approx_fast(rr, dd)
                        nc.vector.tensor_copy(rrb, rr)
                        for b in range(B):
                            eng = nc.gpsimd if b == 3 else nc.vector
                            eng.tensor_tensor(e2[b], e2[b], rrb, op=ALU.mult)
                        for b in range(B):
                            for h in range(2):
                                cs = slice(h * 512, (h + 1) * 512)
                                js = slice(jq * JQ + h * 512, jq * JQ + (h + 1) * 512)
                                nc.tensor.matmul(ps_y[b][h],
                                                 v1p[b][:, it * C:(it + 1) * C],
                                                 f1[b][it][:, js],
                                                 start=False, stop=False)
                                nc.tensor.matmul(ps_y[b][h],
                                                 v_T[b][:, it * C:(it + 1) * C],
                                                 e2[b][:, cs],
                                                 start=False,
                                                 stop=(it == NIT - 1))

                    for b in range(B):
                        out_sb = p_out.tile([C, JQ], dt.float32)
                        for h in range(2):
                            cs = slice(h * 512, (h + 1) * 512)
                            nc.vector.tensor_copy(out_sb[:, cs], ps_y[b][h])
                        nc.sync.dma_start(y_part.ap()[b][:, jsl], out_sb)

    nc.compile()
    return nc


@functools.lru_cache(maxsize=1)
def _get_program():
    return _build_program()


def _prep_inputs(inputs):
    x = np.asarray(inputs["x"], np.float32).reshape(B, C, N)
    ones = np.ones((B, 1, N), np.float32)
    x_ext = np.concatenate([x, ones], axis=1).astype(BF16)          # [B,65,N]

    qw = np.asarray(inputs["qw"], np.float32)
    qb = np.asarray(inputs["qb"], np.float32)
    kw = np.asarray(inputs["kw"], np.float32)
    kb = np.asarray(inputs["kb"], np.float32)
    mw = np.asarray(inputs["mw"], np.float32)
    mb = np.asarray(inputs["mb"], np.float32)
    vw = np.asarray(inputs["vw"], np.float32)
    vb = np.asarray(inputs["vb"], np.float32)
    ww = np.asarray(inputs["ww"], np.float32)
    wb = np.asarray(inputs["wb"], np.float32)
    g = np.asarray(inputs["bn_gamma"], np.float32)
    be = np.asarray(inputs["bn_beta"], np.float32)
    rm = np.asarray(inputs["bn_rm"], np.float32)
    rv = np.asarray(inputs["bn_rv"], np.float32)

    kT = np.concatenate([kw.T, kb[None, :]], axis=0)                # [65,64]
    qg = np.concatenate([qw, qb[:, None]], axis=1)                  # [64,65]
    vT = np.concatenate([vw.T, vb[None, :]], axis=0)                # [65,64]

    inv = g / np.sqrt(rv + EPS)
    wT = np.zeros((C + 1, C), np.float32)
    wT[:C, :] = (ww * inv[:, None]).T / N_CORES
    wT[C, :] = (wb * inv + be - rm * inv) / N_CORES

    xu = x.mean(axis=2)                                             # [B,C]
    negku = -(xu @ kw.T + kb)                                       # [B,C]

    m = np.einsum('c,bcj->bj', mw[0], x) + mb[0]                    # [B,N]

    common = {
        "x_ext": x_ext,
        "kT": kT.astype(BF16),
        "qg": qg.astype(BF16),
        "vT": vT.astype(BF16),
        "wT": wT.astype(BF16),
        "negku": np.ascontiguousarray(negku.T),
        "md": m.astype(BF16),
    }
    in_maps = []
    for ic in range(N_CORES):
        mm = dict(common)
        mm["xsl_ext"] = np.ascontiguousarray(x_ext[:, :, ic * SL:(ic + 1) * SL])
        msl = m[:, ic * SL:(ic + 1) * SL].reshape(B, NIT, 128)      # [B,NIT,128]
        mm["msl"] = np.ascontiguousarray(msl.transpose(2, 0, 1).reshape(128, B * NIT)).astype(np.float32)
        in_maps.append(mm)
    return in_maps


def kernel(**inputs):
    from concourse.bass_utils import run_bass_kernel_spmd

    nc = _get_program()
    in_maps = _prep_inputs(inputs)
    res = run_bass_kernel_spmd(nc, in_maps, core_ids=list(range(N_CORES)))
    y = np.zeros((B, C, N), np.float32)
    for r in res.results:
        y += r["y_part"]
    return y.reshape(B, C, H, W)


if __name__ == "__main__":
    rng = np.random.default_rng(0)
    ins = {
        "x": rng.standard_normal((B, C, H, W), dtype=np.float32),
        "qw": rng.standard_normal((C, C), dtype=np.float32) * 0.05,
        "qb": rng.standard_normal((C,), dtype=np.float32) * 0.05,
        "kw": rng.standard_normal((C, C), dtype=np.float32) * 0.05,
        "kb": rng.standard_normal((C,), dtype=np.float32) * 0.05,
        "mw": rng.standard_normal((1, C), dtype=np.float32) * 0.05,
        "mb": rng.standard_normal((1,), dtype=np.float32) * 0.05,
        "vw": rng.standard_normal((C, C), dtype=np.float32) * 0.05,
        "vb": rng.standard_normal((C,), dtype=np.float32) * 0.05,
        "ww": rng.standard_normal((C, C), dtype=np.float32) * 0.05,
        "wb": rng.standard_normal((C,), dtype=np.float32) * 0.05,
        "bn_gamma": np.ones((C,), np.float32),
        "bn_beta": np.zeros((C,), np.float32),
        "bn_rm": np.zeros((C,), np.float32),
        "bn_rv": np.ones((C,), np.float32),
    }
    out = kernel(**ins)
    print("kernel output", out.shape, out.dtype, np.abs(out).mean())


# revision 8
# speedup vs baseline: 1.1719x; 1.1719x over previous
"""Trainium2 Bass kernel for the non-local-attention block (nn_DNL_74234214744693).

Reference computation (B=4, C=64, H=W=64, N=H*W=4096):
    k = conv1x1(x,kw,kb); k_wh = k - mean_j(k)
    q = conv1x1(x,qw,qb); q_wh = q - mean_j(q)
    qk[b,i,j] = sum_c k_wh[b,c,i] q_wh[b,c,j]
    m  = conv1x1(x,mw,mb) -> [B,N];  mm[b,i,j] = m[b,i]*m[b,j]
    f  = softmax(qk, axis=-1) + softmax(mm, axis=0)   # second softmax over BATCH
    y  = einsum('bci,bij->bcj', v, f) + BN(conv1x1(x,ww,wb))

Key algebraic facts used:
  * softmax_j(k_whT q_wh) == softmax_j(k_whT q_raw): the q-mean term is constant
    along j's softmax rows, so only k needs whitening.
  * q-conv fusion: qk[i,j] = sum_c' g[c',i] x_ext[c',j] with
    g = (qw|qb)^T k_wh  (65 x SL), x_ext having a trailing ones row.
    This removes the full q conv + its PSUM->SBUF copies entirely.
  * softmax_j normalizer Z1[i] indexes the contraction dim, so y1 = (v/Z1) @ e1.
  * batch softmax: f2[b] = e2_b * R with e2_b = exp(m_b_i m_b_j), R = 1/sum_b e2_b.
  * m and the k-mean are data-independent 1-D convs -> computed on the host.

Sharding: each of 8 cores owns a 512-row i-slice of the [N,N] maps for ALL 4
batch samples (exp work is perfectly balanced, no duplication, no collectives).
Each core emits a partial y [4,64,4096]; host sums the 8 partials.
The conv+BN residual is folded into the output matmul with weights pre-scaled
by 1/8 (so the host-side sum reconstructs it exactly once).
"""

import functools

import numpy as np
import ml_dtypes

N_CORES = 8
B, C, H, W = 4, 64, 64, 64
N = H * W                 # 4096
SL = N // N_CORES         # 512  rows of the attention map per core
NIT = SL // 128           # 4    128-row tiles per core
NJQ = 4                   # 1024-wide column blocks in phase B
JQ = N // NJQ             # 1024
EPS = 1e-5

BF16 = ml_dtypes.bfloat16


def _build_program():
    import concourse.bass as bass
    import concourse.tile as tile
    from concourse import bacc, mybir

    dt = mybir.dt
    AF = mybir.ActivationFunctionType
    ALU = mybir.AluOpType
    AX = mybir.AxisListType

    nc = bacc.Bacc("TRN2", target_bir_lowering=False, debug=False,
                   enable_asserts=False, num_devices=1)

    # ---------------- DRAM I/O ----------------
    x_ext = nc.dram_tensor("x_ext", [B, C + 1, N], dt.bfloat16, kind="ExternalInput")
    xsl_ext = nc.dram_tensor("xsl_ext", [B, C + 1, SL], dt.bfloat16, kind="ExternalInput")
    kT = nc.dram_tensor("kT", [C + 1, C], dt.bfloat16, kind="ExternalInput")
    qg = nc.dram_tensor("qg", [C, C + 1], dt.bfloat16, kind="ExternalInput")
    vT = nc.dram_tensor("vT", [C + 1, C], dt.bfloat16, kind="ExternalInput")
    wT = nc.dram_tensor("wT", [C + 1, C], dt.bfloat16, kind="ExternalInput")
    negku = nc.dram_tensor("negku", [C, B], dt.float32, kind="ExternalInput")
    msl = nc.dram_tensor("msl", [128, B * NIT], dt.float32, kind="ExternalInput")
    md = nc.dram_tensor("md", [B, N], dt.bfloat16, kind="ExternalInput")
    y_part = nc.dram_tensor("y_part", [B, C, N], dt.float32, kind="ExternalOutput")

    with tile.TileContext(nc) as tc:
        from contextlib import ExitStack

        with ExitStack() as top:
            # ---------- persistent pools ----------
            consts = top.enter_context(tc.tile_pool(name="consts", bufs=1))
            p_vT = top.enter_context(tc.tile_pool(name="p_vT", bufs=B))
            p_v1p = top.enter_context(tc.tile_pool(name="p_v1p", bufs=B))
            p_f1 = top.enter_context(tc.tile_pool(name="p_f1", bufs=B * NIT))

            sb_kT = consts.tile([C + 1, C], dt.bfloat16)
            sb_qg = consts.tile([C, C + 1], dt.bfloat16)
            sb_vT = consts.tile([C + 1, C], dt.bfloat16)
            sb_wT = consts.tile([C + 1, C], dt.bfloat16)
            sb_negku = consts.tile([C, B], dt.float32)
            sb_msl = consts.tile([128, B * NIT], dt.float32)
            nc.sync.dma_start(sb_kT, kT.ap())
            nc.sync.dma_start(sb_qg, qg.ap())
            nc.sync.dma_start(sb_vT, vT.ap())
            nc.sync.dma_start(sb_wT, wT.ap())
            nc.sync.dma_start(sb_negku, negku.ap())
            nc.sync.dma_start(sb_msl, msl.ap())

            # v_T[b][:, it*64:(it+1)*64] is the [128 i, 64 c] tile for row-tile it
            v_T = [p_vT.tile([128, NIT * C], dt.bfloat16, name=f"v_T{b}", tag="v_T") for b in range(B)]
            v1p = [p_v1p.tile([128, NIT * C], dt.bfloat16, name=f"v1p{b}", tag="v1p") for b in range(B)]
            f1 = [[p_f1.tile([128, N], dt.bfloat16, name=f"f1_{b}_{i}", tag="f1") for i in range(NIT)] for b in range(B)]

            # ---------- phase A: per-b convs (tiny) + qk + e1 ----------
            with ExitStack() as ph0:
                p_x = ph0.enter_context(tc.tile_pool(name="p_x", bufs=2))
                p_xsl = ph0.enter_context(tc.tile_pool(name="p_xsl", bufs=2))
                p_kwh = ph0.enter_context(tc.tile_pool(name="p_kwh", bufs=2))
                p_g = ph0.enter_context(tc.tile_pool(name="p_g", bufs=2))
                psP = ph0.enter_context(tc.tile_pool(name="psP", bufs=2, space="PSUM"))
                p_z = ph0.enter_context(tc.tile_pool(name="p_z", bufs=8))

                def dma_phase(b):
                    x_sb = p_x.tile([C + 1, N], dt.bfloat16, name=f"x_sb{b}", tag="x_sb")
                    xsl_sb = p_xsl.tile([C + 1, SL], dt.bfloat16, name=f"xsl_sb{b}", tag="xsl_sb")
                    nc.sync.dma_start(x_sb, x_ext.ap()[b])
                    nc.sync.dma_start(xsl_sb, xsl_ext.ap()[b])
                    return x_sb, xsl_sb

                def conv_phase(b, x_sb, xsl_sb):
                    # one packed psum tile: g' [65,512] | k [64,512] | v_T [128,256]
                    pc = psP.tile([128, 2048], dt.float32, name=f"pc{b}", tag="psP")
                    k_wh = p_kwh.tile([C, SL], dt.bfloat16, name=f"k_wh{b}", tag="k_wh")
                    g_sb = p_g.tile([C + 1, SL], dt.bfloat16, name=f"g_sb{b}", tag="g_sb")

                    nc.tensor.matmul(pc[0:C, 512:1024], sb_kT, xsl_sb,
                                     start=True, stop=True)
                    nc.vector.tensor_scalar(k_wh, pc[0:C, 512:1024],
                                            scalar1=sb_negku[:, b:b + 1],
                                            scalar2=None, op0=ALU.add)
                    nc.tensor.matmul(pc[0:C + 1, 0:512], sb_qg, k_wh,
                                     start=True, stop=True)
                    nc.vector.tensor_copy(g_sb, pc[0:C + 1, 0:512])
                    for it in range(NIT):
                        nc.tensor.matmul(pc[:, 1024 + it * C:1024 + (it + 1) * C],
                                         xsl_sb[:, it * 128:(it + 1) * 128],
                                         sb_vT, start=True, stop=True)
                    nc.vector.tensor_copy(v_T[b], pc[:, 1024:1024 + NIT * C])
                    return k_wh, g_sb

                def qk_phase(b, g_sb, x_sb, its):
                    for it in its:
                        zp = [p_z.tile([128, 1], dt.float32, name=f"zp{j}", tag="zp") for j in range(2)]
                        for jh in range(2):
                            ps_qk = psP.tile([128, 2048], dt.float32, name="ps_qk", tag="psP")
                            for k4 in range(4):
                                j0 = jh * 2048 + k4 * 512
                                nc.tensor.matmul(
                                    ps_qk[:, k4 * 512:(k4 + 1) * 512],
                                    g_sb[:, it * 128:(it + 1) * 128],
                                    x_sb[:, j0:j0 + 512],
                                    start=True, stop=True)
                            nc.scalar.activation(
                                f1[b][it][:, jh * 2048:(jh + 1) * 2048],
                                ps_qk, AF.Exp, accum_out=zp[jh])
                        z1 = p_z.tile([128, 1], dt.float32)
                        rz = p_z.tile([128, 1], dt.float32)
                        nc.vector.tensor_tensor(z1, zp[0], zp[1], op=ALU.add)
                        nc.vector.reciprocal_approx_fast(rz, z1)
                        nc.vector.tensor_scalar_mul(
                            v1p[b][:, it * C:(it + 1) * C],
                            v_T[b][:, it * C:(it + 1) * C], rz)

                # per-b: convs then qk; conv(b+1) emitted before qk(b)'s last
                # tile so its psum slot + DVE copies hide under e1 exps.
                dmas_cur = dma_phase(0)
                conv_cur = conv_phase(0, *dmas_cur)
                for b in range(B):
                    x_cur = dmas_cur[0]
                    if b + 1 < B:
                        dmas_next = dma_phase(b + 1)
                        qk_phase(b, conv_cur[1], x_cur, range(NIT - 1))
                        conv_next = conv_phase(b + 1, *dmas_next)
                        qk_phase(b, conv_cur[1], x_cur, [NIT - 1])
                        conv_cur = conv_next
                        dmas_cur = dmas_next
                    else:
                        qk_phase(b, conv_cur[1], x_cur, range(NIT))

            # ---------- phase B: e2/D/R/f2 + output matmuls ----------
            with ExitStack() as phB:
                psY = phB.enter_context(tc.tile_pool(name="psY", bufs=8, space="PSUM"))
                p_mbc = phB.enter_context(tc.tile_pool(name="p_mbc", bufs=5))
                p_e2 = phB.enter_context(tc.tile_pool(name="p_e2", bufs=10))
                p_s = phB.enter_context(tc.tile_pool(name="p_s", bufs=2))
                p_dr = phB.enter_context(tc.tile_pool(name="p_dr", bufs=2))
                p_rr = phB.enter_context(tc.tile_pool(name="p_rr", bufs=2))
                p_rb = phB.enter_context(tc.tile_pool(name="p_rb", bufs=3))
                p_xw = phB.enter_context(tc.tile_pool(name="p_xw", bufs=3))
                p_out = phB.enter_context(tc.tile_pool(name="p_out", bufs=2))

                def chain(m_bc, it):
                    # e2_b = exp(m_i * m_j); D = sum_b e2; f2_b = e2_b / D
                    e2 = [p_e2.tile([128, JQ], dt.bfloat16, name=f"e2_{b}", tag="e2") for b in range(B)]
                    for b in range(B):
                        nc.scalar.activation(e2[b], m_bc[b], AF.Exp,
                                             scale=sb_msl[:, b * NIT + it:b * NIT + it + 1])
                    s12 = p_s.tile([128, JQ], dt.bfloat16, tag="s12")
                    s34 = p_s.tile([128, JQ], dt.bfloat16, tag="s34")
                    dd = p_dr.tile([128, JQ], dt.float32, tag="dd")
                    rr = p_rr.tile([128, JQ], dt.float32, tag="rr")
                    rrb = p_rb.tile([128, JQ], dt.bfloat16, tag="rrb")
                    nc.vector.tensor_tensor(s12, e2[0], e2[1], op=ALU.add)
                    nc.vector.tensor_tensor(s34, e2[2], e2[3], op=ALU.add)
                    nc.vector.tensor_tensor(dd, s12, s34, op=ALU.add)
                    nc.vector.reciprocal_approx_fast(rr, dd)
                    nc.vector.tensor_copy(rrb, rr)
                    for b in range(B):
                        eng = nc.gpsimd if b >= 2 else nc.vector
                        eng.tensor_tensor(e2[b], e2[b], rrb, op=ALU.mult)
                    return e2

                for jq in range(NJQ):
                    jsl = slice(jq * JQ, (jq + 1) * JQ)
                    m_bc = []
                    for b in range(B):
                        t = p_mbc.tile([128, JQ], dt.bfloat16, name="m_bc", tag="m_bc")
                        nc.sync.dma_start(t, md.ap()[b:b + 1, jsl].to_broadcast([128, JQ]))
                        m_bc.append(t)
                    x_wx = []
                    for b in range(B):
                        t = p_xw.tile([C + 1, JQ], dt.bfloat16, name="x_wx", tag="x_wx")
                        nc.sync.dma_start(t, x_ext.ap()[b][:, jsl])
                        x_wx.append(t)

                    # f2 chains for it=0,1 start while PE chews the f1 block
                    f2 = [None] * NIT
                    f2[0] = chain(m_bc, 0)
                    f2[1] = chain(m_bc, 1)

                    ps_y = [[psY.tile([C, 512], dt.float32, name=f"ps_y{b}_{h}", tag="ps_y")
                             for h in range(2)] for b in range(B)]
                    # wx residual opens each accumulation group; the f1 block
                    # is a long dense run of PE work with no f2 dependency.
                    for b in range(B):
                        for h in range(2):
                            cs = slice(h * 512, (h + 1) * 512)
                            nc.tensor.matmul(ps_y[b][h], sb_wT, x_wx[b][:, cs],
                                             start=True, stop=False)
                    for it in range(NIT):
                        for b in range(B):
                            for h in range(2):
                                js = slice(jq * JQ + h * 512, jq * JQ + (h + 1) * 512)
                                nc.tensor.matmul(ps_y[b][h],
                                                 v1p[b][:, it * C:(it + 1) * C],
                                                 f1[b][it][:, js],
                                                 start=False, stop=False)
                    for it in range(NIT):
                        for b in range(B):
                            for h in range(2):
                                cs = slice(h * 512, (h + 1) * 512)
                                nc.tensor.matmul(ps_y[b][h],
                                                 v_T[b][:, it * C:(it + 1) * C],
                                                 f2[it][b][:, cs],
                                                 start=False,
                                                 stop=(it == NIT - 1))
                        if it + 2 < NIT:
                            f2[it + 2] = chain(m_bc, it + 2)

                    for b in range(B):
                        out_sb = p_out.tile([C, JQ], dt.float32)
                        for h in range(2):
                            cs = slice(h * 512, (h + 1) * 512)
                            nc.scalar.copy(out_sb[:, cs], ps_y[b][h])
                        nc.sync.dma_start(y_part.ap()[b][:, jsl], out_sb)

    nc.compile()
    return nc


@functools.lru_cache(maxsize=1)
def _get_program():
    return _build_program()


def _prep_inputs(inputs):
    x = np.asarray(inputs["x"], np.float32).reshape(B, C, N)
    ones = np.ones((B, 1, N), np.float32)
    x_ext = np.concatenate([x, ones], axis=1).astype(BF16)          # [B,65,N]

    qw = np.asarray(inputs["qw"], np.float32)
    qb = np.asarray(inputs["qb"], np.float32)
    kw = np.asarray(inputs["kw"], np.float32)
    kb = np.asarray(inputs["kb"], np.float32)
    mw = np.asarray(inputs["mw"], np.float32)
    mb = np.asarray(inputs["mb"], np.float32)
    vw = np.asarray(inputs["vw"], np.float32)
    vb = np.asarray(inputs["vb"], np.float32)
    ww = np.asarray(inputs["ww"], np.float32)
    wb = np.asarray(inputs["wb"], np.float32)
    g = np.asarray(inputs["bn_gamma"], np.float32)
    be = np.asarray(inputs["bn_beta"], np.float32)
    rm = np.asarray(inputs["bn_rm"], np.float32)
    rv = np.asarray(inputs["bn_rv"], np.float32)

    kT = np.concatenate([kw.T, kb[None, :]], axis=0)                # [65,64]
    qg = np.concatenate([qw, qb[:, None]], axis=1)                  # [64,65]
    vT = np.concatenate([vw.T, vb[None, :]], axis=0)                # [65,64]

    inv = g / np.sqrt(rv + EPS)
    wT = np.zeros((C + 1, C), np.float32)
    wT[:C, :] = (ww * inv[:, None]).T / N_CORES
    wT[C, :] = (wb * inv + be - rm * inv) / N_CORES

    xu = x.mean(axis=2)                                             # [B,C]
    negku = -(xu @ kw.T + kb)                                       # [B,C]

    m = np.einsum('c,bcj->bj', mw[0], x) + mb[0]                    # [B,N]

    common = {
        "x_ext": x_ext,
        "kT": kT.astype(BF16),
        "qg": qg.astype(BF16),
        "vT": vT.astype(BF16),
        "wT": wT.astype(BF16),
        "negku": np.ascontiguousarray(negku.T),
        "md": m.astype(BF16),
    }
    in_maps = []
    for ic in range(N_CORES):
        mm = dict(common)
        mm["xsl_ext"] = np.ascontiguousarray(x_ext[:, :, ic * SL:(ic + 1) * SL])
        msl = m[:, ic * SL:(ic + 1) * SL].reshape(B, NIT, 128)      # [B,NIT,128]
        mm["msl"] = np.ascontiguousarray(msl.transpose(2, 0, 1).reshape(128, B * NIT)).astype(np.float32)
        in_maps.append(mm)
    return in_maps


def kernel(**inputs):
    from concourse.bass_utils import run_bass_kernel_spmd

    nc = _get_program()
    in_maps = _prep_inputs(inputs)
    res = run_bass_kernel_spmd(nc, in_maps, core_ids=list(range(N_CORES)))
    y = np.zeros((B, C, N), np.float32)
    for r in res.results:
        y += r["y_part"]
    return y.reshape(B, C, H, W)


if __name__ == "__main__":
    rng = np.random.default_rng(0)
    ins = {
        "x": rng.standard_normal((B, C, H, W), dtype=np.float32),
        "qw": rng.standard_normal((C, C), dtype=np.float32) * 0.05,
        "qb": rng.standard_normal((C,), dtype=np.float32) * 0.05,
        "kw": rng.standard_normal((C, C), dtype=np.float32) * 0.05,
        "kb": rng.standard_normal((C,), dtype=np.float32) * 0.05,
        "mw": rng.standard_normal((1, C), dtype=np.float32) * 0.05,
        "mb": rng.standard_normal((1,), dtype=np.float32) * 0.05,
        "vw": rng.standard_normal((C, C), dtype=np.float32) * 0.05,
        "vb": rng.standard_normal((C,), dtype=np.float32) * 0.05,
        "ww": rng.standard_normal((C, C), dtype=np.float32) * 0.05,
        "wb": rng.standard_normal((C,), dtype=np.float32) * 0.05,
        "bn_gamma": np.ones((C,), np.float32),
        "bn_beta": np.zeros((C,), np.float32),
        "bn_rm": np.zeros((C,), np.float32),
        "bn_rv": np.ones((C,), np.float32),
    }
    out = kernel(**ins)
    print("kernel output", out.shape, out.dtype, np.abs(out).mean())


# revision 12
# speedup vs baseline: 1.1762x; 1.0037x over previous
"""Trainium2 Bass kernel for the non-local-attention block (nn_DNL_74234214744693).

Reference computation (B=4, C=64, H=W=64, N=H*W=4096):
    k = conv1x1(x,kw,kb); k_wh = k - mean_j(k)
    q = conv1x1(x,qw,qb); q_wh = q - mean_j(q)
    qk[b,i,j] = sum_c k_wh[b,c,i] q_wh[b,c,j]
    m  = conv1x1(x,mw,mb) -> [B,N];  mm[b,i,j] = m[b,i]*m[b,j]
    f  = softmax(qk, axis=-1) + softmax(mm, axis=0)   # second softmax over BATCH
    y  = einsum('bci,bij->bcj', v, f) + BN(conv1x1(x,ww,wb))

Key algebraic facts used:
  * softmax_j(k_whT q_wh) == softmax_j(k_whT q_raw): the q-mean term is constant
    along j's softmax rows, so only k needs whitening.
  * q-conv fusion: qk[i,j] = sum_c' g[c',i] x_ext[c',j] with
    g = (qw|qb)^T k_wh  (65 x SL), x_ext having a trailing ones row.
    This removes the full q conv + its PSUM->SBUF copies entirely.
  * softmax_j normalizer Z1[i] indexes the contraction dim, so y1 = (v/Z1) @ e1.
  * batch softmax: f2[b] = e2_b * R with e2_b = exp(m_b_i m_b_j), R = 1/sum_b e2_b.
  * m and the k-mean are data-independent 1-D convs -> computed on the host.

Sharding: each of 8 cores owns a 512-row i-slice of the [N,N] maps for ALL 4
batch samples (exp work is perfectly balanced, no duplication, no collectives).
Each core emits a partial y [4,64,4096]; host sums the 8 partials.
The conv+BN residual is folded into the output matmul with weights pre-scaled
by 1/8 (so the host-side sum reconstructs it exactly once).
"""

import functools

import numpy as np
import ml_dtypes

N_CORES = 8
B, C, H, W = 4, 64, 64, 64
N = H * W                 # 4096
SL = N // N_CORES         # 512  rows of the attention map per core
NIT = SL // 128           # 4    128-row tiles per core
NJQ = 4                   # 1024-wide column blocks in phase B
JQ = N // NJQ             # 1024
EPS = 1e-5

BF16 = ml_dtypes.bfloat16


def _build_program():
    import concourse.bass as bass
    import concourse.tile as tile
    from concourse import bacc, mybir

    dt = mybir.dt
    AF = mybir.ActivationFunctionType
    ALU = mybir.AluOpType
    AX = mybir.AxisListType

    nc = bacc.Bacc("TRN2", target_bir_lowering=False, debug=False,
                   enable_asserts=False, num_devices=1)

    # ---------------- DRAM I/O ----------------
    x_ext = nc.dram_tensor("x_ext", [B, C + 1, N], dt.bfloat16, kind="ExternalInput")
    xsl_ext = nc.dram_tensor("xsl_ext", [B, C + 1, SL], dt.bfloat16, kind="ExternalInput")
    kT = nc.dram_tensor("kT", [C + 1, C], dt.bfloat16, kind="ExternalInput")
    qg = nc.dram_tensor("qg", [C, C + 1], dt.bfloat16, kind="ExternalInput")
    vT = nc.dram_tensor("vT", [C + 1, C], dt.bfloat16, kind="ExternalInput")
    wT = nc.dram_tensor("wT", [C + 1, C], dt.bfloat16, kind="ExternalInput")
    negku = nc.dram_tensor("negku", [C, B], dt.float32, kind="ExternalInput")
    msl = nc.dram_tensor("msl", [128, B * NIT], dt.float32, kind="ExternalInput")
    md = nc.dram_tensor("md", [B, N], dt.bfloat16, kind="ExternalInput")
    y_part = nc.dram_tensor("y_part", [B, C, N], dt.float32, kind="ExternalOutput")

    with tile.TileContext(nc) as tc:
        from contextlib import ExitStack

        with ExitStack() as top:
            # ---------- persistent pools ----------
            consts = top.enter_context(tc.tile_pool(name="consts", bufs=1))
            p_vT = top.enter_context(tc.tile_pool(name="p_vT", bufs=B))
            p_v1p = top.enter_context(tc.tile_pool(name="p_v1p", bufs=B))
            p_f1 = top.enter_context(tc.tile_pool(name="p_f1", bufs=B * NIT))

            sb_kT = consts.tile([C + 1, C], dt.bfloat16)
            sb_qg = consts.tile([C, C + 1], dt.bfloat16)
            sb_vT = consts.tile([C + 1, C], dt.bfloat16)
            sb_wT = consts.tile([C + 1, C], dt.bfloat16)
            sb_negku = consts.tile([C, B], dt.float32)
            sb_msl = consts.tile([128, B * NIT], dt.float32)
            nc.sync.dma_start(sb_kT, kT.ap())
            nc.sync.dma_start(sb_qg, qg.ap())
            nc.sync.dma_start(sb_vT, vT.ap())
            nc.sync.dma_start(sb_wT, wT.ap())
            nc.sync.dma_start(sb_negku, negku.ap())
            nc.sync.dma_start(sb_msl, msl.ap())

            # v_T[b][:, it*64:(it+1)*64] is the [128 i, 64 c] tile for row-tile it
            v_T = [p_vT.tile([128, NIT * C], dt.bfloat16, name=f"v_T{b}", tag="v_T") for b in range(B)]
            v1p = [p_v1p.tile([128, NIT * C], dt.bfloat16, name=f"v1p{b}", tag="v1p") for b in range(B)]
            f1 = [[p_f1.tile([128, N], dt.bfloat16, name=f"f1_{b}_{i}", tag="f1") for i in range(NIT)] for b in range(B)]

            # ---------- phase-B elementwise pools, hoisted so the first two
            # e2 chains can run during the initial x DMA ----------
            p_mbc = top.enter_context(tc.tile_pool(name="p_mbc", bufs=5))
            p_e2 = top.enter_context(tc.tile_pool(name="p_e2", bufs=8))
            p_s = top.enter_context(tc.tile_pool(name="p_s", bufs=2))
            p_dr = top.enter_context(tc.tile_pool(name="p_dr", bufs=1))
            p_rr = top.enter_context(tc.tile_pool(name="p_rr", bufs=1))
            p_rb = top.enter_context(tc.tile_pool(name="p_rb", bufs=2))

            def mbc_dma(jq):
                out = []
                for b in range(B):
                    t = p_mbc.tile([128, JQ], dt.bfloat16, name="m_bc", tag="m_bc")
                    nc.sync.dma_start(
                        t, md.ap()[b:b + 1, jq * JQ:(jq + 1) * JQ].to_broadcast([128, JQ]))
                    out.append(t)
                return out

            def chain(m_bc, it):
                # e2_b = exp(m_i * m_j); D = sum_b e2; f2_b = e2_b / D
                e2 = [p_e2.tile([128, JQ], dt.bfloat16, name=f"e2_{b}", tag="e2") for b in range(B)]
                for b in range(B):
                    nc.scalar.activation(e2[b], m_bc[b], AF.Exp,
                                         scale=sb_msl[:, b * NIT + it:b * NIT + it + 1])
                s12 = p_s.tile([128, JQ], dt.bfloat16, tag="s12")
                s34 = p_s.tile([128, JQ], dt.bfloat16, tag="s34")
                dd = p_dr.tile([128, JQ], dt.float32, tag="dd")
                rr = p_rr.tile([128, JQ], dt.float32, tag="rr")
                rrb = p_rb.tile([128, JQ], dt.bfloat16, tag="rrb")
                nc.vector.tensor_tensor(s12, e2[0], e2[1], op=ALU.add)
                nc.vector.tensor_tensor(s34, e2[2], e2[3], op=ALU.add)
                nc.vector.tensor_tensor(dd, s12, s34, op=ALU.add)
                nc.vector.reciprocal_approx_fast(rr, dd)
                nc.vector.tensor_copy(rrb, rr)
                for b in range(B):
                    eng = nc.gpsimd if b >= 2 else nc.vector
                    eng.tensor_tensor(e2[b], e2[b], rrb, op=ALU.mult)
                return e2

            mbc_cur = mbc_dma(0)
            pre = [chain(mbc_cur, 0), chain(mbc_cur, 1)]

            # ---------- phase A: per-b convs (tiny) + qk + e1 ----------
            NXC = 4                      # x held as 4 column chunks of 1024
            with ExitStack() as ph0:
                p_x = ph0.enter_context(tc.tile_pool(name="p_x", bufs=2 * NXC))
                p_xsl = ph0.enter_context(tc.tile_pool(name="p_xsl", bufs=2))
                p_kwh = ph0.enter_context(tc.tile_pool(name="p_kwh", bufs=2))
                p_g = ph0.enter_context(tc.tile_pool(name="p_g", bufs=2))
                psP = ph0.enter_context(tc.tile_pool(name="psP", bufs=2, space="PSUM"))
                p_z = ph0.enter_context(tc.tile_pool(name="p_z", bufs=8))

                def dma_phase(b):
                    # xsl first (unblocks the convs), then x in column chunks
                    # so the first qk matmuls start ~3us after the DMA begins.
                    xsl_sb = p_xsl.tile([C + 1, SL], dt.bfloat16, name=f"xsl_sb{b}", tag="xsl_sb")
                    nc.sync.dma_start(xsl_sb, xsl_ext.ap()[b])
                    xch = []
                    for cxi in range(NXC):
                        t = p_x.tile([C + 1, N // NXC], dt.bfloat16, name=f"x_sb{b}_{cxi}", tag="x_sb")
                        nc.sync.dma_start(t, x_ext.ap()[b][:, cxi * (N // NXC):(cxi + 1) * (N // NXC)])
                        xch.append(t)
                    return xch, xsl_sb

                def conv_phase(b, x_sb, xsl_sb):
                    # one packed psum tile: g' [65,512] | k [64,512] | v_T [128,256]
                    pc = psP.tile([128, 2048], dt.float32, name=f"pc{b}", tag="psP")
                    k_wh = p_kwh.tile([C, SL], dt.bfloat16, name=f"k_wh{b}", tag="k_wh")
                    g_sb = p_g.tile([C + 1, SL], dt.bfloat16, name=f"g_sb{b}", tag="g_sb")

                    nc.tensor.matmul(pc[0:C, 512:1024], sb_kT, xsl_sb,
                                     start=True, stop=True)
                    nc.vector.tensor_scalar(k_wh, pc[0:C, 512:1024],
                                            scalar1=sb_negku[:, b:b + 1],
                                            scalar2=None, op0=ALU.add)
                    nc.tensor.matmul(pc[0:C + 1, 0:512], sb_qg, k_wh,
                                     start=True, stop=True)
                    nc.vector.tensor_copy(g_sb, pc[0:C + 1, 0:512])
                    for it in range(NIT):
                        nc.tensor.matmul(pc[:, 1024 + it * C:1024 + (it + 1) * C],
                                         xsl_sb[:, it * 128:(it + 1) * 128],
                                         sb_vT, start=True, stop=True)
                    nc.vector.tensor_copy(v_T[b], pc[:, 1024:1024 + NIT * C])
                    return k_wh, g_sb

                def qk_phase(b, g_sb, xch, its):
                    for it in its:
                        zp = [p_z.tile([128, 1], dt.float32, name=f"zp{j}", tag="zp") for j in range(2)]
                        for jh in range(2):
                            ps_qk = psP.tile([128, 2048], dt.float32, name="ps_qk", tag="psP")
                            for k4 in range(4):
                                j0 = jh * 2048 + k4 * 512
                                nc.tensor.matmul(
                                    ps_qk[:, k4 * 512:(k4 + 1) * 512],
                                    g_sb[:, it * 128:(it + 1) * 128],
                                    xch[j0 // 1024][:, (j0 % 1024):(j0 % 1024) + 512],
                                    start=True, stop=True)
                            nc.scalar.activation(
                                f1[b][it][:, jh * 2048:(jh + 1) * 2048],
                                ps_qk, AF.Exp, accum_out=zp[jh])
                        z1 = p_z.tile([128, 1], dt.float32)
                        rz = p_z.tile([128, 1], dt.float32)
                        nc.vector.tensor_tensor(z1, zp[0], zp[1], op=ALU.add)
                        nc.vector.reciprocal_approx_fast(rz, z1)
                        nc.vector.tensor_scalar_mul(
                            v1p[b][:, it * C:(it + 1) * C],
                            v_T[b][:, it * C:(it + 1) * C], rz)

                # per-b: convs then qk; conv(b+1) emitted before qk(b)'s last
                # tile so its psum slot + DVE copies hide under e1 exps.
                dmas_cur = dma_phase(0)
                conv_cur = conv_phase(0, *dmas_cur)
                for b in range(B):
                    x_cur = dmas_cur[0]
                    if b + 1 < B:
                        dmas_next = dma_phase(b + 1)
                        qk_phase(b, conv_cur[1], x_cur, range(NIT - 1))
                        conv_next = conv_phase(b + 1, *dmas_next)
                        qk_phase(b, conv_cur[1], x_cur, [NIT - 1])
                        conv_cur = conv_next
                        dmas_cur = dmas_next
                    else:
                        qk_phase(b, conv_cur[1], x_cur, range(NIT))

            # ---------- phase B: f1/f2 apply + wx, with e2 chains threaded ----------
            with ExitStack() as phB:
                psY = phB.enter_context(tc.tile_pool(name="psY", bufs=8, space="PSUM"))
                p_xw = phB.enter_context(tc.tile_pool(name="p_xw", bufs=3))
                p_out = phB.enter_context(tc.tile_pool(name="p_out", bufs=4))

                def f1_mm(ps_y, jq, it):
                    for b in range(B):
                        for h in range(2):
                            js = slice(jq * JQ + h * 512, jq * JQ + (h + 1) * 512)
                            nc.tensor.matmul(ps_y[b][h],
                                             v1p[b][:, it * C:(it + 1) * C],
                                             f1[b][it][:, js],
                                             start=False, stop=False)

                def f2_mm(ps_y, f2t, it):
                    for b in range(B):
                        for h in range(2):
                            cs = slice(h * 512, (h + 1) * 512)
                            nc.tensor.matmul(ps_y[b][h],
                                             v_T[b][:, it * C:(it + 1) * C],
                                             f2t[b][:, cs],
                                             start=False,
                                             stop=(it == NIT - 1))

                for jq in range(NJQ):
                    jsl = slice(jq * JQ, (jq + 1) * JQ)
                    m_bc = mbc_cur
                    f2 = [pre[0], pre[1], None, None]
                    x_wx = []
                    for b in range(B):
                        t = p_xw.tile([C + 1, JQ], dt.bfloat16, name="x_wx", tag="x_wx")
                        nc.sync.dma_start(t, x_ext.ap()[b][:, jsl])
                        x_wx.append(t)

                    ps_y = [[psY.tile([C, 512], dt.float32, name=f"ps_y{b}_{h}", tag="ps_y")
                             for h in range(2)] for b in range(B)]
                    # wx opens each accumulation group; f1/f2 applies are
                    # interleaved so e2 tiles free steadily for the chains.
                    for b in range(B):
                        for h in range(2):
                            cs = slice(h * 512, (h + 1) * 512)
                            nc.tensor.matmul(ps_y[b][h], sb_wT, x_wx[b][:, cs],
                                             start=True, stop=False)
                    f1_mm(ps_y, jq, 0)
                    f1_mm(ps_y, jq, 1)
                    f2_mm(ps_y, f2[0], 0)
                    f2[2] = chain(m_bc, 2)
                    f1_mm(ps_y, jq, 2)
                    f2_mm(ps_y, f2[1], 1)
                    f2[3] = chain(m_bc, 3)
                    f1_mm(ps_y, jq, 3)
                    f2_mm(ps_y, f2[2], 2)
                    if jq + 1 < NJQ:
                        mbc_next = mbc_dma(jq + 1)
                        pre0_next = chain(mbc_next, 0)
                    f2_mm(ps_y, f2[3], 3)

                    # out copies: ACT takes b=0,1, DVE b=2,3; emitted after the
                    # next jq's first chain so ACT/DVE keep streaming exps
                    # while the accumulation groups close.
                    for b in range(B):
                        out_sb = p_out.tile([C, 512], dt.float32)
                        out_sb2 = p_out.tile([C, 512], dt.float32)
                        if b < 2:
                            nc.scalar.copy(out_sb, ps_y[b][0])
                            nc.scalar.copy(out_sb2, ps_y[b][1])
                        else:
                            nc.vector.tensor_copy(out_sb, ps_y[b][0])
                            nc.vector.tensor_copy(out_sb2, ps_y[b][1])
                        nc.sync.dma_start(y_part.ap()[b][:, jq * JQ:jq * JQ + 512], out_sb)
                        nc.sync.dma_start(y_part.ap()[b][:, jq * JQ + 512:(jq + 1) * JQ], out_sb2)

                    if jq + 1 < NJQ:
                        pre = [pre0_next, chain(mbc_next, 1)]
                        mbc_cur = mbc_next

    nc.compile()
    return nc


@functools.lru_cache(maxsize=1)
def _get_program():
    return _build_program()


def _prep_inputs(inputs):
    x = np.asarray(inputs["x"], np.float32).reshape(B, C, N)
    ones = np.ones((B, 1, N), np.float32)
    x_ext = np.concatenate([x, ones], axis=1).astype(BF16)          # [B,65,N]

    qw = np.asarray(inputs["qw"], np.float32)
    qb = np.asarray(inputs["qb"], np.float32)
    kw = np.asarray(inputs["kw"], np.float32)
    kb = np.asarray(inputs["kb"], np.float32)
    mw = np.asarray(inputs["mw"], np.float32)
    mb = np.asarray(inputs["mb"], np.float32)
    vw = np.asarray(inputs["vw"], np.float32)
    vb = np.asarray(inputs["vb"], np.float32)
    ww = np.asarray(inputs["ww"], np.float32)
    wb = np.asarray(inputs["wb"], np.float32)
    g = np.asarray(inputs["bn_gamma"], np.float32)
    be = np.asarray(inputs["bn_beta"], np.float32)
    rm = np.asarray(inputs["bn_rm"], np.float32)
    rv = np.asarray(inputs["bn_rv"], np.float32)

    kT = np.concatenate([kw.T, kb[None, :]], axis=0)                # [65,64]
    qg = np.concatenate([qw, qb[:, None]], axis=1)                  # [64,65]
    vT = np.concatenate([vw.T, vb[None, :]], axis=0)                # [65,64]

    inv = g / np.sqrt(rv + EPS)
    wT = np.zeros((C + 1, C), np.float32)
    wT[:C, :] = (ww * inv[:, None]).T / N_CORES
    wT[C, :] = (wb * inv + be - rm * inv) / N_CORES

    xu = x.mean(axis=2)                                             # [B,C]
    negku = -(xu @ kw.T + kb)                                       # [B,C]

    m = np.einsum('c,bcj->bj', mw[0], x) + mb[0]                    # [B,N]

    common = {
        "x_ext": x_ext,
        "kT": kT.astype(BF16),
        "qg": qg.astype(BF16),
        "vT": vT.astype(BF16),
        "wT": wT.astype(BF16),
        "negku": np.ascontiguousarray(negku.T),
        "md": m.astype(BF16),
    }
    in_maps = []
    for ic in range(N_CORES):
        mm = dict(common)
        mm["xsl_ext"] = np.ascontiguousarray(x_ext[:, :, ic * SL:(ic + 1) * SL])
        msl = m[:, ic * SL:(ic + 1) * SL].reshape(B, NIT, 128)      # [B,NIT,128]
        mm["msl"] = np.ascontiguousarray(msl.transpose(2, 0, 1).reshape(128, B * NIT)).astype(np.float32)
        in_maps.append(mm)
    return in_maps


def kernel(**inputs):
    from concourse.bass_utils import run_bass_kernel_spmd

    nc = _get_program()
    in_maps = _prep_inputs(inputs)
    res = run_bass_kernel_spmd(nc, in_maps, core_ids=list(range(N_CORES)))
    y = np.zeros((B, C, N), np.float32)
    for r in res.results:
        y += r["y_part"]
    return y.reshape(B, C, H, W)


if __name__ == "__main__":
    rng = np.random.default_rng(0)
    ins = {
        "x": rng.standard_normal((B, C, H, W), dtype=np.float32),
        "qw": rng.standard_normal((C, C), dtype=np.float32) * 0.05,
        "qb": rng.standard_normal((C,), dtype=np.float32) * 0.05,
        "kw": rng.standard_normal((C, C), dtype=np.float32) * 0.05,
        "kb": rng.standard_normal((C,), dtype=np.float32) * 0.05,
        "mw": rng.standard_normal((1, C), dtype=np.float32) * 0.05,
        "mb": rng.standard_normal((1,), dtype=np.float32) * 0.05,
        "vw": rng.standard_normal((C, C), dtype=np.float32) * 0.05,
        "vb": rng.standard_normal((C,), dtype=np.float32) * 0.05,
        "ww": rng.standard_normal((C, C), dtype=np.float32) * 0.05,
        "wb": rng.standard_normal((C,), dtype=np.float32) * 0.05,
        "bn_gamma": np.ones((C,), np.float32),
        "bn_beta": np.zeros((C,), np.float32),
        "bn_rm": np.zeros((C,), np.float32),
        "bn_rv": np.ones((C,), np.float32),
    }
    out = kernel(**ins)
    print("kernel output", out.shape, out.dtype, np.abs(out).mean())


# revision 15
# speedup vs baseline: 1.2872x; 1.0943x over previous
"""Trainium2 Bass kernel for the non-local-attention block (nn_DNL_74234214744693).

Reference computation (B=4, C=64, H=W=64, N=H*W=4096):
    k = conv1x1(x,kw,kb); k_wh = k - mean_j(k)
    q = conv1x1(x,qw,qb); q_wh = q - mean_j(q)
    qk[b,i,j] = sum_c k_wh[b,c,i] q_wh[b,c,j]
    m  = conv1x1(x,mw,mb) -> [B,N];  mm[b,i,j] = m[b,i]*m[b,j]
    f  = softmax(qk, axis=-1) + softmax(mm, axis=0)   # second softmax over BATCH
    y  = einsum('bci,bij->bcj', v, f) + BN(conv1x1(x,ww,wb))

Key algebraic facts used:
  * softmax_j(k_whT q_wh) == softmax_j(k_whT q_raw): the q-mean term is constant
    along j's softmax rows, so only k needs whitening.
  * q-conv fusion: qk[i,j] = sum_c' g[c',i] x_ext[c',j] with
    g = (qw|qb)^T k_wh  (65 x SL), x_ext having a trailing ones row.
    This removes the full q conv + its PSUM->SBUF copies entirely.
  * softmax_j normalizer Z1[i] indexes the contraction dim, so y1 = (v/Z1) @ e1.
  * batch softmax: f2[b] = e2_b * R with e2_b = exp(m_b_i m_b_j), R = 1/sum_b e2_b.
  * m and the k-mean are data-independent 1-D convs -> computed on the host.

Sharding: each of 8 cores owns a 512-row i-slice of the [N,N] maps for ALL 4
batch samples (exp work is perfectly balanced, no duplication, no collectives).
Each core emits a partial y [4,64,4096]; host sums the 8 partials.
The conv+BN residual is folded into the output matmul with weights pre-scaled
by 1/8 (so the host-side sum reconstructs it exactly once).
"""

import functools

import numpy as np
import ml_dtypes

N_CORES = 8
B, C, H, W = 4, 64, 64, 64
N = H * W                 # 4096
SL = N // N_CORES         # 512  rows of the attention map per core
NIT = SL // 128           # 4    128-row tiles per core
NJQ = 4                   # 1024-wide column blocks in phase B
JQ = N // NJQ             # 1024
EPS = 1e-5

BF16 = ml_dtypes.bfloat16


def _build_program():
    import concourse.bass as bass
    import concourse.tile as tile
    from concourse import bacc, mybir

    dt = mybir.dt
    AF = mybir.ActivationFunctionType
    ALU = mybir.AluOpType
    AX = mybir.AxisListType

    nc = bacc.Bacc("TRN2", target_bir_lowering=False, debug=False,
                   enable_asserts=False, num_devices=1)

    # ---------------- DRAM I/O ----------------
    x_ext = nc.dram_tensor("x_ext", [B, C + 1, N], dt.bfloat16, kind="ExternalInput")
    xsl_ext = nc.dram_tensor("xsl_ext", [B, C + 1, SL], dt.bfloat16, kind="ExternalInput")
    kT = nc.dram_tensor("kT", [C + 1, C], dt.bfloat16, kind="ExternalInput")
    qg = nc.dram_tensor("qg", [C, C + 1], dt.bfloat16, kind="ExternalInput")
    vT = nc.dram_tensor("vT", [C + 1, C], dt.bfloat16, kind="ExternalInput")
    wT = nc.dram_tensor("wT", [C + 1, C], dt.bfloat16, kind="ExternalInput")
    negku = nc.dram_tensor("negku", [C, B], dt.float32, kind="ExternalInput")
    msl = nc.dram_tensor("msl", [128, B * NIT], dt.float32, kind="ExternalInput")
    md = nc.dram_tensor("md", [B, N], dt.bfloat16, kind="ExternalInput")
    y_part = nc.dram_tensor("y_part", [B, C, N], dt.float32, kind="ExternalOutput")

    with tile.TileContext(nc) as tc:
        from contextlib import ExitStack

        with ExitStack() as top:
            # ---------- persistent pools ----------
            consts = top.enter_context(tc.tile_pool(name="consts", bufs=1))
            p_vT = top.enter_context(tc.tile_pool(name="p_vT", bufs=B))
            p_v1p = top.enter_context(tc.tile_pool(name="p_v1p", bufs=B))
            p_f1 = top.enter_context(tc.tile_pool(name="p_f1", bufs=B * NIT))

            sb_kT = consts.tile([C + 1, C], dt.bfloat16)
            sb_qg = consts.tile([C, C + 1], dt.bfloat16)
            sb_vT = consts.tile([C + 1, C], dt.bfloat16)
            sb_wT = consts.tile([C + 1, C], dt.bfloat16)
            sb_negku = consts.tile([C, B], dt.float32)
            sb_msl = consts.tile([128, B * NIT], dt.float32)
            # msl first: the jq=0 e2 chains only need msl + their m_bc DMA,
            # so ACT starts ~1us in instead of waiting for the x DMA.
            nc.sync.dma_start(sb_msl, msl.ap())

            # v_T[b][:, it*64:(it+1)*64] is the [128 i, 64 c] tile for row-tile it
            v_T = [p_vT.tile([128, NIT * C], dt.bfloat16, name=f"v_T{b}", tag="v_T") for b in range(B)]
            v1p = [p_v1p.tile([128, NIT * C], dt.bfloat16, name=f"v1p{b}", tag="v1p") for b in range(B)]
            f1 = [[p_f1.tile([128, N], dt.bfloat16, name=f"f1_{b}_{i}", tag="f1") for i in range(NIT)] for b in range(B)]

            # ---------- phase-B elementwise pools, hoisted so the first two
            # e2 chains can run during the initial x DMA ----------
            p_mbc = top.enter_context(tc.tile_pool(name="p_mbc", bufs=4))
            p_e2 = top.enter_context(tc.tile_pool(name="p_e2", bufs=12))
            p_s = top.enter_context(tc.tile_pool(name="p_s", bufs=2))
            p_dr = top.enter_context(tc.tile_pool(name="p_dr", bufs=1))
            p_rr = top.enter_context(tc.tile_pool(name="p_rr", bufs=1))
            p_rb = top.enter_context(tc.tile_pool(name="p_rb", bufs=2))

            def mbc_dma(jq):
                out = []
                for b in range(B):
                    t = p_mbc.tile([128, JQ], dt.bfloat16, name="m_bc", tag="m_bc")
                    nc.sync.dma_start(
                        t, md.ap()[b:b + 1, jq * JQ:(jq + 1) * JQ].to_broadcast([128, JQ]))
                    out.append(t)
                return out

            def chain(m_bc, it):
                # e2_b = exp(m_i * m_j); D = sum_b e2; f2_b = e2_b / D
                e2 = [p_e2.tile([128, JQ], dt.bfloat16, name=f"e2_{b}", tag="e2") for b in range(B)]
                for b in range(B):
                    nc.scalar.activation(e2[b], m_bc[b], AF.Exp,
                                         scale=sb_msl[:, b * NIT + it:b * NIT + it + 1])
                s12 = p_s.tile([128, JQ], dt.bfloat16, tag="s12")
                s34 = p_s.tile([128, JQ], dt.bfloat16, tag="s34")
                dd = p_dr.tile([128, JQ], dt.float32, tag="dd")
                rr = p_rr.tile([128, JQ], dt.float32, tag="rr")
                rrb = p_rb.tile([128, JQ], dt.bfloat16, tag="rrb")
                nc.vector.tensor_tensor(s12, e2[0], e2[1], op=ALU.add)
                nc.vector.tensor_tensor(s34, e2[2], e2[3], op=ALU.add)
                nc.vector.tensor_tensor(dd, s12, s34, op=ALU.add)
                nc.vector.reciprocal_approx_fast(rr, dd)
                nc.vector.tensor_copy(rrb, rr)
                for b in range(B):
                    eng = nc.gpsimd if b >= 2 else nc.vector
                    eng.tensor_tensor(e2[b], e2[b], rrb, op=ALU.mult)
                return e2

            mbc_cur = mbc_dma(0)
            nc.sync.dma_start(sb_kT, kT.ap())
            nc.sync.dma_start(sb_qg, qg.ap())
            nc.sync.dma_start(sb_vT, vT.ap())
            nc.sync.dma_start(sb_wT, wT.ap())
            nc.sync.dma_start(sb_negku, negku.ap())
            pre = [chain(mbc_cur, 0), chain(mbc_cur, 1)]

            # ---------- phase A: per-b convs (tiny) + qk + e1 ----------
            NXC = 4                      # x held as 4 column chunks of 1024
            with ExitStack() as ph0:
                p_x = ph0.enter_context(tc.tile_pool(name="p_x", bufs=6))
                p_xsl = ph0.enter_context(tc.tile_pool(name="p_xsl", bufs=2))
                p_kwh = ph0.enter_context(tc.tile_pool(name="p_kwh", bufs=2))
                p_g = ph0.enter_context(tc.tile_pool(name="p_g", bufs=2))
                psP = ph0.enter_context(tc.tile_pool(name="psP", bufs=2, space="PSUM"))
                p_z = ph0.enter_context(tc.tile_pool(name="p_z", bufs=8))

                def dma_phase(b):
                    # xsl first (unblocks the convs), then x in column chunks
                    # so the first qk matmuls start ~3us after the DMA begins.
                    xsl_sb = p_xsl.tile([C + 1, SL], dt.bfloat16, name=f"xsl_sb{b}", tag="xsl_sb")
                    nc.sync.dma_start(xsl_sb, xsl_ext.ap()[b])
                    xch = []
                    for cxi in range(NXC):
                        t = p_x.tile([C + 1, N // NXC], dt.bfloat16, name=f"x_sb{b}_{cxi}", tag="x_sb")
                        nc.sync.dma_start(t, x_ext.ap()[b][:, cxi * (N // NXC):(cxi + 1) * (N // NXC)])
                        xch.append(t)
                    return xch, xsl_sb

                def conv_phase(b, x_sb, xsl_sb):
                    # one packed psum tile: g' [65,512] | k [64,512] | v_T [128,256]
                    pc = psP.tile([128, 2048], dt.float32, name=f"pc{b}", tag="psP")
                    k_wh = p_kwh.tile([C, SL], dt.bfloat16, name=f"k_wh{b}", tag="k_wh")
                    g_sb = p_g.tile([C + 1, SL], dt.bfloat16, name=f"g_sb{b}", tag="g_sb")

                    nc.tensor.matmul(pc[0:C, 512:1024], sb_kT, xsl_sb,
                                     start=True, stop=True)
                    nc.vector.tensor_scalar(k_wh, pc[0:C, 512:1024],
                                            scalar1=sb_negku[:, b:b + 1],
                                            scalar2=None, op0=ALU.add)
                    nc.tensor.matmul(pc[0:C + 1, 0:512], sb_qg, k_wh,
                                     start=True, stop=True)
                    nc.vector.tensor_copy(g_sb, pc[0:C + 1, 0:512])
                    for it in range(NIT):
                        nc.tensor.matmul(pc[:, 1024 + it * C:1024 + (it + 1) * C],
                                         xsl_sb[:, it * 128:(it + 1) * 128],
                                         sb_vT, start=True, stop=True)
                    nc.vector.tensor_copy(v_T[b], pc[:, 1024:1024 + NIT * C])
                    return k_wh, g_sb

                def qk_phase(b, g_sb, xch, its):
                    for it in its:
                        zp = [p_z.tile([128, 1], dt.float32, name=f"zp{j}", tag="zp") for j in range(2)]
                        for jh in range(2):
                            ps_qk = psP.tile([128, 2048], dt.float32, name="ps_qk", tag="psP")
                            for k4 in range(4):
                                j0 = jh * 2048 + k4 * 512
                                nc.tensor.matmul(
                                    ps_qk[:, k4 * 512:(k4 + 1) * 512],
                                    g_sb[:, it * 128:(it + 1) * 128],
                                    xch[j0 // 1024][:, (j0 % 1024):(j0 % 1024) + 512],
                                    start=True, stop=True)
                            nc.scalar.activation(
                                f1[b][it][:, jh * 2048:(jh + 1) * 2048],
                                ps_qk, AF.Exp, accum_out=zp[jh])
                        z1 = p_z.tile([128, 1], dt.float32)
                        rz = p_z.tile([128, 1], dt.float32)
                        nc.vector.tensor_tensor(z1, zp[0], zp[1], op=ALU.add)
                        nc.vector.reciprocal_approx_fast(rz, z1)
                        nc.vector.tensor_scalar_mul(
                            v1p[b][:, it * C:(it + 1) * C],
                            v_T[b][:, it * C:(it + 1) * C], rz)

                # per-b: convs then qk; conv(b+1) emitted before qk(b)'s last
                # tile so its psum slot + DVE copies hide under e1 exps.
                dmas_cur = dma_phase(0)
                conv_cur = conv_phase(0, *dmas_cur)
                for b in range(B):
                    x_cur = dmas_cur[0]
                    if b + 1 < B:
                        dmas_next = dma_phase(b + 1)
                        qk_phase(b, conv_cur[1], x_cur, range(2))
                        conv_next = conv_phase(b + 1, *dmas_next)
                        qk_phase(b, conv_cur[1], x_cur, range(2, NIT))
                        conv_cur = conv_next
                        dmas_cur = dmas_next
                    else:
                        qk_phase(b, conv_cur[1], x_cur, range(NIT))

            # ---------- phase B: f1/f2 apply + wx, with e2 chains threaded ----------
            with ExitStack() as phB:
                psY = phB.enter_context(tc.tile_pool(name="psY", bufs=8, space="PSUM"))
                p_xw = phB.enter_context(tc.tile_pool(name="p_xw", bufs=2))
                p_out = phB.enter_context(tc.tile_pool(name="p_out", bufs=2))

                def f1_mm(ps_y, jq, it):
                    for b in range(B):
                        for h in range(2):
                            js = slice(jq * JQ + h * 512, jq * JQ + (h + 1) * 512)
                            nc.tensor.matmul(ps_y[b][h],
                                             v1p[b][:, it * C:(it + 1) * C],
                                             f1[b][it][:, js],
                                             start=False, stop=False)

                def f2_mm(ps_y, f2t, it):
                    for b in range(B):
                        for h in range(2):
                            cs = slice(h * 512, (h + 1) * 512)
                            nc.tensor.matmul(ps_y[b][h],
                                             v_T[b][:, it * C:(it + 1) * C],
                                             f2t[b][:, cs],
                                             start=False,
                                             stop=(it == NIT - 1))

                for jq in range(NJQ):
                    jsl = slice(jq * JQ, (jq + 1) * JQ)
                    m_bc = mbc_cur
                    f2 = [pre[0], pre[1], None, None]
                    x_wx = []
                    for b in range(B):
                        t = p_xw.tile([C + 1, JQ], dt.bfloat16, name="x_wx", tag="x_wx")
                        nc.sync.dma_start(t, x_ext.ap()[b][:, jsl])
                        x_wx.append(t)

                    ps_y = [[psY.tile([C, 512], dt.float32, name=f"ps_y{b}_{h}", tag="ps_y")
                             for h in range(2)] for b in range(B)]
                    # wx opens each accumulation group; f1/f2 applies are
                    # interleaved so e2 tiles free steadily for the chains.
                    for b in range(B):
                        for h in range(2):
                            cs = slice(h * 512, (h + 1) * 512)
                            nc.tensor.matmul(ps_y[b][h], sb_wT, x_wx[b][:, cs],
                                             start=True, stop=False)
                    f1_mm(ps_y, jq, 0)
                    f1_mm(ps_y, jq, 1)
                    f2[2] = chain(m_bc, 2)
                    f2_mm(ps_y, f2[0], 0)
                    f1_mm(ps_y, jq, 2)
                    f2[3] = chain(m_bc, 3)
                    f2_mm(ps_y, f2[1], 1)
                    f1_mm(ps_y, jq, 3)
                    f2_mm(ps_y, f2[2], 2)
                    if jq + 1 < NJQ:
                        mbc_next = mbc_dma(jq + 1)
                        pre0_next = chain(mbc_next, 0)
                    f2_mm(ps_y, f2[3], 3)

                    # out copies: ACT takes b=0,1, DVE b=2,3; emitted after the
                    # next jq's first chain so ACT/DVE keep streaming exps
                    # while the accumulation groups close.
                    for b in range(B):
                        out_sb = p_out.tile([C, JQ], dt.float32)
                        if b < 2:
                            nc.scalar.copy(out_sb[:, 0:512], ps_y[b][0])
                            nc.scalar.copy(out_sb[:, 512:JQ], ps_y[b][1])
                        else:
                            nc.vector.tensor_copy(out_sb[:, 0:512], ps_y[b][0])
                            nc.vector.tensor_copy(out_sb[:, 512:JQ], ps_y[b][1])
                        nc.sync.dma_start(y_part.ap()[b][:, jsl], out_sb)

                    if jq + 1 < NJQ:
                        pre = [pre0_next, chain(mbc_next, 1)]
                        mbc_cur = mbc_next

    nc.compile()
    return nc


@functools.lru_cache(maxsize=1)
def _get_program():
    return _build_program()


def _prep_inputs(inputs):
    x = np.asarray(inputs["x"], np.float32).reshape(B, C, N)
    ones = np.ones((B, 1, N), np.float32)
    x_ext = np.concatenate([x, ones], axis=1).astype(BF16)          # [B,65,N]

    qw = np.asarray(inputs["qw"], np.float32)
    qb = np.asarray(inputs["qb"], np.float32)
    kw = np.asarray(inputs["kw"], np.float32)
    kb = np.asarray(inputs["kb"], np.float32)
    mw = np.asarray(inputs["mw"], np.float32)
    mb = np.asarray(inputs["mb"], np.float32)
    vw = np.asarray(inputs["vw"], np.float32)
    vb = np.asarray(inputs["vb"], np.float32)
    ww = np.asarray(inputs["ww"], np.float32)
    wb = np.asarray(inputs["wb"], np.float32)
    g = np.asarray(inputs["bn_gamma"], np.float32)
    be = np.asarray(inputs["bn_beta"], np.float32)
    rm = np.asarray(inputs["bn_rm"], np.float32)
    rv = np.asarray(inputs["bn_rv"], np.float32)

    kT = np.concatenate([kw.T, kb[None, :]], axis=0)                # [65,64]
    qg = np.concatenate([qw, qb[:, None]], axis=1)                  # [64,65]
    vT = np.concatenate([vw.T, vb[None, :]], axis=0)                # [65,64]

    inv = g / np.sqrt(rv + EPS)
    wT = np.zeros((C + 1, C), np.float32)
    wT[:C, :] = (ww * inv[:, None]).T / N_CORES
    wT[C, :] = (wb * inv + be - rm * inv) / N_CORES

    xu = x.mean(axis=2)                                             # [B,C]
    negku = -(xu @ kw.T + kb)                                       # [B,C]

    m = np.einsum('c,bcj->bj', mw[0], x) + mb[0]                    # [B,N]

    common = {
        "x_ext": x_ext,
        "kT": kT.astype(BF16),
        "qg": qg.astype(BF16),
        "vT": vT.astype(BF16),
        "wT": wT.astype(BF16),
        "negku": np.ascontiguousarray(negku.T),
        "md": m.astype(BF16),
    }
    in_maps = []
    for ic in range(N_CORES):
        mm = dict(common)
        mm["xsl_ext"] = np.ascontiguousarray(x_ext[:, :, ic * SL:(ic + 1) * SL])
        msl = m[:, ic * SL:(ic + 1) * SL].reshape(B, NIT, 128)      # [B,NIT,128]
        mm["msl"] = np.ascontiguousarray(msl.transpose(2, 0, 1).reshape(128, B * NIT)).astype(np.float32)
        in_maps.append(mm)
    return in_maps


def kernel(**inputs):
    from concourse.bass_utils import run_bass_kernel_spmd

    nc = _get_program()
    in_maps = _prep_inputs(inputs)
    res = run_bass_kernel_spmd(nc, in_maps, core_ids=list(range(N_CORES)))
    y = np.zeros((B, C, N), np.float32)
    for r in res.results:
        y += r["y_part"]
    return y.reshape(B, C, H, W)


if __name__ == "__main__":
    rng = np.random.default_rng(0)
    ins = {
        "x": rng.standard_normal((B, C, H, W), dtype=np.float32),
        "qw": rng.standard_normal((C, C), dtype=np.float32) * 0.05,
        "qb": rng.standard_normal((C,), dtype=np.float32) * 0.05,
        "kw": rng.standard_normal((C, C), dtype=np.float32) * 0.05,
        "kb": rng.standard_normal((C,), dtype=np.float32) * 0.05,
        "mw": rng.standard_normal((1, C), dtype=np.float32) * 0.05,
        "mb": rng.standard_normal((1,), dtype=np.float32) * 0.05,
        "vw": rng.standard_normal((C, C), dtype=np.float32) * 0.05,
        "vb": rng.standard_normal((C,), dtype=np.float32) * 0.05,
        "ww": rng.standard_normal((C, C), dtype=np.float32) * 0.05,
        "wb": rng.standard_normal((C,), dtype=np.float32) * 0.05,
        "bn_gamma": np.ones((C,), np.float32),
        "bn_beta": np.zeros((C,), np.float32),
        "bn_rm": np.zeros((C,), np.float32),
        "bn_rv": np.ones((C,), np.float32),
    }
    out = kernel(**ins)
    print("kernel output", out.shape, out.dtype, np.abs(out).mean())
